# revision 25
# baseline (speedup 1.0000x reference)
"""Trainium2 Bass kernel for a dense multi-head attention layer.

Problem (hardcoded, self-contained):
  query [4, 2048, 1024] f32, key/value [4, 2048, 1024] f32,
  Wq/Wk/Wv/Wo [1024, 1024] f32, bq/bk/bv/bo [1024] f32.
  out = softmax((q Wq + bq)(k Wk + bk)^T / 8) (v Wv + bv) Wo + bo
  with 16 heads of dim 64.

Sharding: 8 cores = 4 batches x 2 query-T halves (pure data parallel, no
collectives). Each core computes a full [1024, 1024] output slice; the host
concatenates.

Per-core dataflow (bf16 matmuls, fp32 PSUM accumulation, fp32 softmax):
  - raw inputs cast f32->bf16 during SWDGE DMA into SBUF 128-token stages
    (order: q, k, v so each phase's consumer can chase its producer), then
    transposed to feature-major tiles on TensorE (identity matmul).
  - weights ride the parallel HWDGE (sync) queue as raw f32 kc-slices and
    are cast to bf16 on VectorE, so the serial SWDGE descriptor generator
    carries only the activation stages.
  - Qt = Wq^T q^T, Kt = Wk^T k^T (feature-major); V = (v^T)^T Wv token-major
    into a ones-augmented [V_h | 1] layout. bq/bk fused into PSUM eviction.
    Kt lives in a 3-deep rotating pool of per-chunk [128, S] tiles: chunk j
    is produced during pair j-1's attention and dies after pair j.
  - attention per head-pair (two heads share a 128-partition chunk) per
    512-column t-chunk: scores St[s,t] = Kt_h^T Qt_h as K=64 row-paired
    matmuls (heads in row groups 0-63/64-127), softmax without
    max-subtraction (scores are O(6) by construction): P = exp(St/8) on
    ScalarE straight out of PSUM. PV: O'_h = [V_h|1]^T P accumulated over
    s-tiles; row 64 of O' is the softmax denominator l[t].
  - deferred PE work is interleaved into attention steps (up to 2 fillers
    per step) to keep the PE dense while ScalarE works on exp:
    pairs 0-3 carry the V-projection for heads 8-15 (4 s-tiles per pair)
    plus K-projection chunk j+1; pairs 4-6 carry K chunks 5-7.
  - normalize O = O'[0:64]/l + bv: l is broadcast across partitions via a
    DRAM bounce on the HWDGE queue, reciprocal_approx_fast in place, one
    TT multiply.
  - out = OT^T Wo + bo (bo pre-broadcast via DMA, fused in eviction), f32,
    written back over HWDGE.
"""

import numpy as np

import concourse.bass as bass
import concourse.bacc as bacc
import concourse.mybir as mybir
import concourse.tile as tile

F32 = mybir.dt.float32
BF16 = mybir.dt.bfloat16
EXP = mybir.ActivationFunctionType.Exp

P = 128  # partitions
HD = 64  # head dim


class Cfg:
    def __init__(self, T, S, D, H):
        self.T = T  # query tokens per core
        self.S = S  # kv tokens
        self.D = D  # model dim
        self.H = H  # heads
        assert D == H * HD
        self.KC = D // P          # contraction chunks of 128
        self.PAIRS = H // 2       # head pairs
        self.ST = S // P          # s tiles of 128
        self.TC = min(512, T)     # matmul free-dim chunk over t
        self.NT = T // self.TC    # t chunks
        self.DC = min(512, D)     # matmul free-dim chunk over d
        self.ND = D // self.DC    # d chunks
        self.SC = min(512, S)     # matmul free-dim chunk over s
        self.NS = S // self.SC    # s chunks (for K proj)
        self.AW = 2 * self.TC     # scores/acc tile width (2 heads x t-chunk)


FULL = Cfg(T=1024, S=2048, D=1024, H=16)
N_CORES = 8


def _pbcast(ap, n, drop_first=True):
    """Broadcast an AP across n partitions (step-0 partition dim)."""
    dims = [list(d) for d in list(ap.ap)]
    if drop_first:
        assert dims[0][1] == 1, dims
        dims = dims[1:]
    return bass.AP(tensor=ap.tensor, offset=ap.offset, ap=[[0, n]] + dims)


def build_kernel(ctx, tc, cfg, io):
    nc = tc.nc
    c = cfg
    scale = 1.0 / np.sqrt(HD)

    dram = ctx.enter_context(tc.tile_pool(name="dram", bufs=1, space="DRAM"))
    consts = ctx.enter_context(tc.tile_pool(name="consts", bufs=1))
    wpool = ctx.enter_context(tc.tile_pool(name="w", bufs=2))
    wstage = ctx.enter_context(tc.tile_pool(name="wstage", bufs=2))
    rawpool = ctx.enter_context(tc.tile_pool(name="raw", bufs=2))
    actpool = ctx.enter_context(tc.tile_pool(name="acts", bufs=1))
    ktpool = ctx.enter_context(tc.tile_pool(name="ktp", bufs=3))
    psum = ctx.enter_context(tc.tile_pool(name="psum", bufs=1, space="PSUM"))
    ppool = ctx.enter_context(tc.tile_pool(name="p", bufs=2))
    npool = ctx.enter_context(tc.tile_pool(name="norm", bufs=2))
    n1pool = ctx.enter_context(tc.tile_pool(name="norm1", bufs=1))
    outpool = ctx.enter_context(tc.tile_pool(name="outsb", bufs=2))

    from concourse.masks import make_identity

    ident = consts.tile([P, P], BF16)
    make_identity(nc, ident)
    stpool = ctx.enter_context(tc.tile_pool(name="stage", bufs=2))

    def load_w(name):
        """Raw f32 kc-halves over HWDGE, cast to bf16 on ScalarE (idle in
        the prologue; VectorE is the prologue bottleneck)."""
        w = wpool.tile([P, c.KC, c.D], BF16, name=f"{name}_sb", tag="w")
        src = io[name][:].rearrange("(c p) n -> p c n", p=P)
        hw = c.D // 2
        for kc in range(c.KC):
            for h in range(2):
                s = wstage.tile([P, hw], F32, tag="wst")
                # alternate the two HWDGE queues to halve serialization
                eng = nc.sync if (2 * kc + h) % 2 == 0 else nc.scalar
                eng.dma_start(out=s[:], in_=src[:, kc, h * hw : (h + 1) * hw])
                nc.scalar.copy(out=w[:, kc, h * hw : (h + 1) * hw], in_=s[:])
        return w

    def load_biases():
        bq_col = consts.tile([P, c.KC], F32)
        nc.gpsimd.dma_start(
            out=bq_col[:], in_=io["bq"][:].rearrange("(c p) -> p c", p=P)
        )
        bk_col = consts.tile([P, c.KC], F32)
        nc.gpsimd.dma_start(
            out=bk_col[:], in_=io["bk"][:].rearrange("(c p) -> p c", p=P)
        )
        bv64 = consts.tile([HD, c.H], F32)
        nc.gpsimd.dma_start(
            out=bv64[:], in_=io["bv"][:].rearrange("(h p) -> p h", p=HD)
        )
        bo_bc = consts.tile([P, c.D], BF16)
        nc.gpsimd.dma_start(
            out=bo_bc[:], in_=_pbcast(io["bo"][:], P, drop_first=False)
        )
        return bq_col, bk_col, bv64, bo_bc

    def transpose_in(dst, src_handle, ntok):
        """Feature-major transpose of a raw input: cast 128-token blocks into
        SBUF on the SWDGE queue, transpose on TensorE via identity matmuls."""
        for tt in range(ntok // P):
            stage = stpool.tile([P, c.D], BF16, tag="stage")
            nc.gpsimd.dma_start(
                out=stage[:], in_=src_handle[tt * P : (tt + 1) * P, :]
            )
            pst = psum.tile([P, c.D], BF16, name=f"tp_{tt}", tag="sc", bufs=2)
            for kc in range(c.KC):
                nc.tensor.transpose(
                    pst[:, kc * P : (kc + 1) * P],
                    stage[:, kc * P : (kc + 1) * P],
                    ident[:],
                )
            # one batched eviction for all 8 transposes of this token block
            nc.vector.tensor_copy(
                out=dst[:, :, tt * P : (tt + 1) * P],
                in_=pst[:].rearrange("p (c q) -> p c q", q=P),
            )

    # PSUM tags: "sc" [128, AW] bufs=2 (4 banks), "acc" [65, AW] (2 banks),
    # "proj" [128, 512] bufs=2 (2 banks) -> exactly 8 banks.
    proj_i = [0]

    def proj_psum(width):
        t = psum.tile([P, width], F32, name=f"ps{proj_i[0]}", tag="proj", bufs=2)
        proj_i[0] += 1
        return t

    # ---- prologue: q stages lead the SWDGE queue; biases follow ----
    Wq_sb = load_w("Wq")
    qT = rawpool.tile([P, c.KC, c.T], BF16, tag="raw")
    transpose_in(qT, io["query"], c.T)
    bq_col, bk_col, bv64, bo_bc = load_biases()

    # ---- Q projection: Qt [d_out, t] feature-major ----
    Qt = actpool.tile([P, c.KC, c.T], BF16, tag="qt")
    for mc in range(c.KC):
        for n in range(c.NT):
            ps = proj_psum(c.TC)
            for kc in range(c.KC):
                nc.tensor.matmul(
                    ps[:],
                    lhsT=Wq_sb[:, kc, mc * P : (mc + 1) * P],
                    rhs=qT[:, kc, n * c.TC : (n + 1) * c.TC],
                    start=(kc == 0),
                    stop=(kc == c.KC - 1),
                )
            nc.vector.tensor_scalar_add(
                out=Qt[:, mc, n * c.TC : (n + 1) * c.TC],
                in0=ps[:],
                scalar1=bq_col[:, mc : mc + 1],
            )

    # ---- K staging + K projection chunk 0 ----
    Wk_sb = load_w("Wk")
    keyT = rawpool.tile([P, c.KC, c.S], BF16, tag="raw")
    transpose_in(keyT, io["key"], c.S)

    kt_tiles = {}

    def kproj_chunk_ops(mc):
        """Yield thunks: the 32 matmuls + evictions for K chunk mc into a
        rotating [P, S] tile."""
        ops = []

        def mk_tile(mc=mc):
            kt_tiles[mc] = ktpool.tile([P, c.S], BF16, name=f"kt_{mc}",
                                       tag="kt")

        for n in range(c.NS):
            ps_holder = []
            for kc in range(c.KC):
                def mm(n=n, kc=kc, mc=mc, ps_holder=ps_holder):
                    if mc not in kt_tiles:
                        mk_tile()
                    if kc == 0:
                        ps_holder.append(proj_psum(c.SC))
                    nc.tensor.matmul(
                        ps_holder[-1][:],
                        lhsT=Wk_sb[:, kc, mc * P : (mc + 1) * P],
                        rhs=keyT[:, kc, n * c.SC : (n + 1) * c.SC],
                        start=(kc == 0),
                        stop=(kc == c.KC - 1),
                    )
                    if kc == c.KC - 1:
                        nc.vector.tensor_scalar_add(
                            out=kt_tiles[mc][:, n * c.SC : (n + 1) * c.SC],
                            in0=ps_holder[-1][:],
                            scalar1=bk_col[:, mc : mc + 1],
                        )
                ops.append(mm)
        return ops

    for op in kproj_chunk_ops(0):
        op()

    # ---- V staging + V projection n=0 (heads 0-7); n=1 deferred ----
    Wv_sb = load_w("Wv")
    valT = rawpool.tile([P, c.KC, c.S], BF16, tag="raw")
    transpose_in(valT, io["value"], c.S)
    vaug = actpool.tile([P, c.ST, c.H, 66], BF16, tag="vaug")
    nc.vector.memset(vaug[:, :, :, 64:65], 1.0)
    hpd = c.DC // HD  # heads per d-chunk

    def vproj_ops(n, sc_list):
        """Thunks: 8 matmuls + eviction per s-tile for V d-chunk n."""
        ops = []
        for sc in sc_list:
            ps_holder = []
            for kc in range(c.KC):
                def mm(n=n, sc=sc, kc=kc, ps_holder=ps_holder):
                    if kc == 0:
                        ps_holder.append(proj_psum(c.DC))
                    nc.tensor.matmul(
                        ps_holder[-1][:],
                        lhsT=valT[:, kc, sc * P : (sc + 1) * P],
                        rhs=Wv_sb[:, kc, n * c.DC : (n + 1) * c.DC],
                        start=(kc == 0),
                        stop=(kc == c.KC - 1),
                    )
                    if kc == c.KC - 1:
                        nc.vector.tensor_copy(
                            out=vaug[:, sc, n * hpd : (n + 1) * hpd, 0:64],
                            in_=ps_holder[-1][:].rearrange(
                                "p (h x) -> p h x", x=HD
                            ),
                        )
                ops.append(mm)
        return ops

    # n=0 blocks are interleaved into pair 0's first t-chunk with a
    # 3-s-tile lookahead (PV of step st needs vaug[st]); nothing is
    # emitted before attention so the window opens right after K chunk 0
    for op in vproj_ops(0, list(range(c.ST))):
        op()

    # ---- attention ----
    OT = actpool.tile([P, c.PAIRS, c.T], BF16, tag="ot")
    for j in range(c.PAIRS):
        pending = []
        if j + 1 < c.KC:
            pending += kproj_chunk_ops(j + 1)
        if j < 4:
            pending += vproj_ops(1, list(range(4 * j, 4 * j + 4)))
        pend_i = 0
        Kt_j = kt_tiles[j]
        for n in range(c.NT):
            acc = psum.tile([65, c.AW], F32, name=f"acc_{j}_{n}", tag="acc")
            for st in range(c.ST):
                sp = psum.tile([P, c.AW], F32, name=f"sc_{j}_{n}_{st}", tag="sc",
                               bufs=2)
                for hh in range(2):
                    po = hh * HD
                    nc.tensor.matmul(
                        sp[:, hh * c.TC : (hh + 1) * c.TC],
                        lhsT=Kt_j[po : po + HD, st * P : (st + 1) * P],
                        rhs=Qt[po : po + HD, j, n * c.TC : (n + 1) * c.TC],
                        start=True,
                        stop=True,
                        tile_position=(po, 0),
                    )
                pt = ppool.tile([P, c.AW], BF16, tag="p")
                nc.scalar.activation(pt[:], sp[:], EXP, scale=float(scale))
                for hh in range(2):
                    sl = slice(hh * c.TC, (hh + 1) * c.TC)
                    nc.tensor.matmul(
                        acc[:, sl],
                        lhsT=vaug[:, st, 2 * j + hh, 0:65],
                        rhs=pt[:, sl],
                        start=(st == 0),
                        stop=(st == c.ST - 1),
                    )
                # interleave deferred projection work, two ops per step
                for _ in range(2):
                    if pend_i < len(pending):
                        pending[pend_i]()
                        pend_i += 1
            # normalization: O = O'[0:64]/l + bv ; l = O'[64]
            nrm = npool.tile([65, c.AW], F32, tag="nrm")
            nc.vector.tensor_copy(out=nrm[:], in_=acc[:])
            l_dram = dram.tile([c.AW], F32, name=f"ld_{j}_{n}", tag="ld", bufs=2)
            nc.sync.dma_start(out=l_dram[:], in_=nrm[64:65, :])
            rv = n1pool.tile([HD, c.AW], F32, name=f"rv_{j}_{n}", tag="rv")
            nc.sync.dma_start(out=rv[:], in_=_pbcast(l_dram[:], HD, drop_first=False))
            nc.vector.reciprocal_approx_fast(out=rv[:], in_=rv[:])
            tmp = n1pool.tile([HD, c.AW], BF16, name=f"tmp_{j}_{n}", tag="tmp")
            nc.vector.tensor_mul(out=tmp[:], in0=nrm[0:64, :], in1=rv[:])
            tsl = slice(n * c.TC, (n + 1) * c.TC)
            nc.vector.tensor_scalar_add(
                out=OT[0:64, j, tsl], in0=tmp[:, 0 : c.TC],
                scalar1=bv64[:, 2 * j : 2 * j + 1],
            )
            shf = n1pool.tile([HD, c.TC], BF16, name=f"shf_{j}_{n}", tag="shf")
            nc.vector.tensor_scalar_add(
                out=shf[:], in0=tmp[:, c.TC : 2 * c.TC],
                scalar1=bv64[:, 2 * j + 1 : 2 * j + 2],
            )
            nc.sync.dma_start(out=OT[64:128, j, tsl], in_=shf[:])
        while pend_i < len(pending):
            pending[pend_i]()
            pend_i += 1

    # ---- output projection (Wo load emitted here so its queue slot sits
    # after the attention bounces; its buffer frees once Wk is dead; casts
    # ride VectorE, which is idle in the tail, instead of ScalarE) ----
    Wo_sb = wpool.tile([P, c.KC, c.D], BF16, name="Wo_sb", tag="w")
    wo_src = io["Wo"][:].rearrange("(c p) n -> p c n", p=P)
    for kc in range(c.KC):
        for h in range(2):
            s = wstage.tile([P, c.D // 2], F32, tag="wst")
            eng = nc.sync if (2 * kc + h) % 2 == 0 else nc.scalar
            eng.dma_start(
                out=s[:],
                in_=wo_src[:, kc, h * (c.D // 2) : (h + 1) * (c.D // 2)],
            )
            nc.vector.tensor_copy(
                out=Wo_sb[:, kc, h * (c.D // 2) : (h + 1) * (c.D // 2)],
                in_=s[:],
            )
    for m in range(c.T // P):
        for n in range(c.ND):
            ps = proj_psum(c.DC)
            for j in range(c.PAIRS):
                nc.tensor.matmul(
                    ps[:],
                    lhsT=OT[:, j, m * P : (m + 1) * P],
                    rhs=Wo_sb[:, j, n * c.DC : (n + 1) * c.DC],
                    start=(j == 0),
                    stop=(j == c.PAIRS - 1),
                )
            osb = outpool.tile([P, c.DC], F32, tag="osb")
            nc.vector.tensor_add(
                out=osb[:], in0=ps[:], in1=bo_bc[:, n * c.DC : (n + 1) * c.DC]
            )
            nc.gpsimd.dma_start(
                out=io["out"][m * P : (m + 1) * P, n * c.DC : (n + 1) * c.DC],
                in_=osb[:],
            )


def build_nc(cfg=FULL):
    from contextlib import ExitStack

    nc = bacc.Bacc()
    io = {
        "query": nc.dram_tensor("query", [cfg.T, cfg.D], F32, kind="ExternalInput"),
        "key": nc.dram_tensor("key", [cfg.S, cfg.D], F32, kind="ExternalInput"),
        "value": nc.dram_tensor("value", [cfg.S, cfg.D], F32, kind="ExternalInput"),
        "Wq": nc.dram_tensor("Wq", [cfg.D, cfg.D], F32, kind="ExternalInput"),
        "Wk": nc.dram_tensor("Wk", [cfg.D, cfg.D], F32, kind="ExternalInput"),
        "Wv": nc.dram_tensor("Wv", [cfg.D, cfg.D], F32, kind="ExternalInput"),
        "Wo": nc.dram_tensor("Wo", [cfg.D, cfg.D], F32, kind="ExternalInput"),
        "bq": nc.dram_tensor("bq", [cfg.D], F32, kind="ExternalInput"),
        "bk": nc.dram_tensor("bk", [cfg.D], F32, kind="ExternalInput"),
        "bv": nc.dram_tensor("bv", [cfg.D], F32, kind="ExternalInput"),
        "bo": nc.dram_tensor("bo", [cfg.D], F32, kind="ExternalInput"),
        "out": nc.dram_tensor("out", [cfg.T, cfg.D], F32, kind="ExternalOutput"),
    }
    with tile.TileContext(nc) as tc:
        with ExitStack() as ctx:
            build_kernel(ctx, tc, cfg, io)
    nc.finalize()
    return nc



def make_in_maps(arr):
    """Per-core input maps for the 4-batch x 2-T-half sharding."""
    B, T_full, D = arr["query"].shape
    half = T_full // 2
    in_maps = []
    for core in range(N_CORES):
        b, h = divmod(core, 2)
        m = {
            "query": np.ascontiguousarray(arr["query"][b, h * half : (h + 1) * half]),
            "key": arr["key"][b],
            "value": arr["value"][b],
        }
        for w in ("Wq", "Wk", "Wv", "Wo", "bq", "bk", "bv", "bo"):
            m[w] = arr[w]
        in_maps.append(m)
    return in_maps


def run(inputs, trace=False):
    from concourse.bass_utils import run_bass_kernel_spmd

    arr = {k: np.ascontiguousarray(np.asarray(v, dtype=np.float32))
           for k, v in inputs.items()}
    B, T_full, D = arr["query"].shape
    half = T_full // 2
    nc = build_nc(FULL)
    in_maps = make_in_maps(arr)
    res = run_bass_kernel_spmd(nc, in_maps, list(range(N_CORES)), trace=trace)
    out = np.empty((B, T_full, D), np.float32)
    for core in range(N_CORES):
        b, h = divmod(core, 2)
        out[b, h * half : (h + 1) * half] = res.results[core]["out"]
    return out, res


def kernel(**inputs):
    out, _ = run(inputs, trace=False)
    return out


# revision 28
# speedup vs baseline: 1.1808x; 1.1808x over previous
"""Trainium2 Bass kernel for a dense multi-head attention layer.

Problem (hardcoded, self-contained):
  query [4, 2048, 1024] f32, key/value [4, 2048, 1024] f32,
  Wq/Wk/Wv/Wo [1024, 1024] f32, bq/bk/bv/bo [1024] f32.
  out = softmax((q Wq + bq)(k Wk + bk)^T / 8) (v Wv + bv) Wo + bo
  with 16 heads of dim 64.

Sharding: 8 cores = 4 batches x 2 query-T halves (pure data parallel, no
collectives). Each core computes a full [1024, 1024] output slice; the host
concatenates.

Per-core dataflow (bf16 matmuls, fp32 PSUM accumulation, fp32 softmax):
  - raw inputs cast f32->bf16 during SWDGE DMA into SBUF 128-token stages
    (order: q, k, v so each phase's consumer can chase its producer), then
    transposed to feature-major tiles on TensorE (identity matmul).
  - weights ride the parallel HWDGE (sync) queue as raw f32 kc-slices and
    are cast to bf16 on VectorE, so the serial SWDGE descriptor generator
    carries only the activation stages.
  - Qt = Wq^T q^T, Kt = Wk^T k^T (feature-major); V = (v^T)^T Wv token-major
    into a ones-augmented [V_h | 1] layout. bq/bk fused into PSUM eviction.
    Kt lives in a 3-deep rotating pool of per-chunk [128, S] tiles: chunk j
    is produced during pair j-1's attention and dies after pair j.
  - attention per head-pair (two heads share a 128-partition chunk) per
    512-column t-chunk: scores St[s,t] = Kt_h^T Qt_h as K=64 row-paired
    matmuls (heads in row groups 0-63/64-127), softmax without
    max-subtraction (scores are O(6) by construction): P = exp(St/8) on
    ScalarE straight out of PSUM. PV: O'_h = [V_h|1]^T P accumulated over
    s-tiles; row 64 of O' is the softmax denominator l[t].
  - deferred PE work is interleaved into attention steps (up to 2 fillers
    per step) to keep the PE dense while ScalarE works on exp:
    pairs 0-3 carry the V-projection for heads 8-15 (4 s-tiles per pair)
    plus K-projection chunk j+1; pairs 4-6 carry K chunks 5-7.
  - normalize O = O'[0:64]/l + bv: l is broadcast across partitions via a
    DRAM bounce on the HWDGE queue, reciprocal_approx_fast in place, one
    TT multiply.
  - out = OT^T Wo + bo (bo pre-broadcast via DMA, fused in eviction), f32,
    written back over HWDGE.
"""

import numpy as np

import concourse.bass as bass
import concourse.bacc as bacc
import concourse.mybir as mybir
import concourse.tile as tile

F32 = mybir.dt.float32
BF16 = mybir.dt.bfloat16
EXP = mybir.ActivationFunctionType.Exp

P = 128  # partitions
HD = 64  # head dim


class Cfg:
    def __init__(self, T, S, D, H):
        self.T = T  # query tokens per core
        self.S = S  # kv tokens
        self.D = D  # model dim
        self.H = H  # heads
        assert D == H * HD
        self.KC = D // P          # contraction chunks of 128
        self.PAIRS = H // 2       # head pairs
        self.ST = S // P          # s tiles of 128
        self.TC = min(512, T)     # matmul free-dim chunk over t
        self.NT = T // self.TC    # t chunks
        self.DC = min(512, D)     # matmul free-dim chunk over d
        self.ND = D // self.DC    # d chunks
        self.SC = min(512, S)     # matmul free-dim chunk over s
        self.NS = S // self.SC    # s chunks (for K proj)
        self.AW = 2 * self.TC     # scores/acc tile width (2 heads x t-chunk)


FULL = Cfg(T=1024, S=2048, D=1024, H=16)
N_CORES = 8


def _pbcast(ap, n, drop_first=True):
    """Broadcast an AP across n partitions (step-0 partition dim)."""
    dims = [list(d) for d in list(ap.ap)]
    if drop_first:
        assert dims[0][1] == 1, dims
        dims = dims[1:]
    return bass.AP(tensor=ap.tensor, offset=ap.offset, ap=[[0, n]] + dims)


def build_kernel(ctx, tc, cfg, io):
    nc = tc.nc
    c = cfg
    scale = 1.0 / np.sqrt(HD)

    dram = ctx.enter_context(tc.tile_pool(name="dram", bufs=1, space="DRAM"))
    consts = ctx.enter_context(tc.tile_pool(name="consts", bufs=1))
    wpool = ctx.enter_context(tc.tile_pool(name="w", bufs=2))
    wstage = ctx.enter_context(tc.tile_pool(name="wstage", bufs=2))
    rawpool = ctx.enter_context(tc.tile_pool(name="raw", bufs=2))
    actpool = ctx.enter_context(tc.tile_pool(name="acts", bufs=1))
    ktpool = ctx.enter_context(tc.tile_pool(name="ktp", bufs=3))
    psum = ctx.enter_context(tc.tile_pool(name="psum", bufs=1, space="PSUM"))
    ppool = ctx.enter_context(tc.tile_pool(name="p", bufs=2))
    npool = ctx.enter_context(tc.tile_pool(name="norm", bufs=2))
    n1pool = ctx.enter_context(tc.tile_pool(name="norm1", bufs=1))
    outpool = ctx.enter_context(tc.tile_pool(name="outsb", bufs=2))

    from concourse.masks import make_identity

    ident = consts.tile([P, P], BF16)
    make_identity(nc, ident)
    stpool = ctx.enter_context(tc.tile_pool(name="stage", bufs=2))

    def load_w(name):
        """Raw f32 kc-halves over HWDGE, cast to bf16 on ScalarE (idle in
        the prologue; VectorE is the prologue bottleneck)."""
        w = wpool.tile([P, c.KC, c.D], BF16, name=f"{name}_sb", tag="w")
        src = io[name][:].rearrange("(c p) n -> p c n", p=P)
        hw = c.D // 2
        for kc in range(c.KC):
            for h in range(2):
                s = wstage.tile([P, hw], F32, tag="wst")
                # alternate the two HWDGE queues to halve serialization
                eng = nc.sync if (2 * kc + h) % 2 == 0 else nc.scalar
                eng.dma_start(out=s[:], in_=src[:, kc, h * hw : (h + 1) * hw])
                nc.scalar.copy(out=w[:, kc, h * hw : (h + 1) * hw], in_=s[:])
        return w

    def load_biases():
        bq_col = consts.tile([P, c.KC], F32)
        nc.gpsimd.dma_start(
            out=bq_col[:], in_=io["bq"][:].rearrange("(c p) -> p c", p=P)
        )
        bk_col = consts.tile([P, c.KC], F32)
        nc.gpsimd.dma_start(
            out=bk_col[:], in_=io["bk"][:].rearrange("(c p) -> p c", p=P)
        )
        bv64 = consts.tile([HD, c.H], F32)
        nc.gpsimd.dma_start(
            out=bv64[:], in_=io["bv"][:].rearrange("(h p) -> p h", p=HD)
        )
        bo_bc = consts.tile([P, c.D], BF16)
        nc.gpsimd.dma_start(
            out=bo_bc[:], in_=_pbcast(io["bo"][:], P, drop_first=False)
        )
        return bq_col, bk_col, bv64, bo_bc

    def transpose_in(dst, src_handle, ntok):
        """Feature-major transpose of a raw input: cast 128-token blocks into
        SBUF on the SWDGE queue, transpose on TensorE via identity matmuls."""
        for tt in range(ntok // P):
            stage = stpool.tile([P, c.D], BF16, tag="stage")
            nc.gpsimd.dma_start(
                out=stage[:], in_=src_handle[tt * P : (tt + 1) * P, :]
            )
            pst = psum.tile([P, c.D], BF16, name=f"tp_{tt}", tag="sc", bufs=2)
            for kc in range(c.KC):
                nc.tensor.transpose(
                    pst[:, kc * P : (kc + 1) * P],
                    stage[:, kc * P : (kc + 1) * P],
                    ident[:],
                )
            # one batched eviction for all 8 transposes of this token block
            nc.vector.tensor_copy(
                out=dst[:, :, tt * P : (tt + 1) * P],
                in_=pst[:].rearrange("p (c q) -> p c q", q=P),
            )

    # PSUM tags: "sc" [128, AW] bufs=2 (4 banks), "acc" [65, AW] (2 banks),
    # "proj" [128, 512] bufs=2 (2 banks) -> exactly 8 banks.
    proj_i = [0]

    def proj_psum(width):
        t = psum.tile([P, width], F32, name=f"ps{proj_i[0]}", tag="proj", bufs=2)
        proj_i[0] += 1
        return t

    # ---- prologue: q stages lead the SWDGE queue; biases follow ----
    Wq_sb = load_w("Wq")
    qT = rawpool.tile([P, c.KC, c.T], BF16, tag="raw")
    transpose_in(qT, io["query"], c.T)
    bq_col, bk_col, bv64, bo_bc = load_biases()

    # ---- Q projection: Qt [d_out, t] feature-major ----
    Qt = actpool.tile([P, c.KC, c.T], BF16, tag="qt")
    for mc in range(c.KC):
        for n in range(c.NT):
            ps = proj_psum(c.TC)
            for kc in range(c.KC):
                nc.tensor.matmul(
                    ps[:],
                    lhsT=Wq_sb[:, kc, mc * P : (mc + 1) * P],
                    rhs=qT[:, kc, n * c.TC : (n + 1) * c.TC],
                    start=(kc == 0),
                    stop=(kc == c.KC - 1),
                )
            nc.vector.tensor_scalar_add(
                out=Qt[:, mc, n * c.TC : (n + 1) * c.TC],
                in0=ps[:],
                scalar1=bq_col[:, mc : mc + 1],
            )

    # ---- K staging + K projection chunk 0 ----
    Wk_sb = load_w("Wk")
    keyT = rawpool.tile([P, c.KC, c.S], BF16, tag="raw")
    transpose_in(keyT, io["key"], c.S)

    kt_tiles = {}

    def kproj_chunk_ops(mc):
        """Yield thunks: the 32 matmuls + evictions for K chunk mc into a
        rotating [P, S] tile."""
        ops = []

        def mk_tile(mc=mc):
            kt_tiles[mc] = ktpool.tile([P, c.S], BF16, name=f"kt_{mc}",
                                       tag="kt")

        for n in range(c.NS):
            ps_holder = []
            for kc in range(c.KC):
                def mm(n=n, kc=kc, mc=mc, ps_holder=ps_holder):
                    if mc not in kt_tiles:
                        mk_tile()
                    if kc == 0:
                        ps_holder.append(proj_psum(c.SC))
                    nc.tensor.matmul(
                        ps_holder[-1][:],
                        lhsT=Wk_sb[:, kc, mc * P : (mc + 1) * P],
                        rhs=keyT[:, kc, n * c.SC : (n + 1) * c.SC],
                        start=(kc == 0),
                        stop=(kc == c.KC - 1),
                    )
                    if kc == c.KC - 1:
                        nc.vector.tensor_scalar_add(
                            out=kt_tiles[mc][:, n * c.SC : (n + 1) * c.SC],
                            in0=ps_holder[-1][:],
                            scalar1=bk_col[:, mc : mc + 1],
                        )
                ops.append(mm)
        return ops

    for op in kproj_chunk_ops(0):
        op()

    # ---- V staging + V projection n=0 (heads 0-7); n=1 deferred ----
    Wv_sb = load_w("Wv")
    valT = rawpool.tile([P, c.KC, c.S], BF16, tag="raw")
    transpose_in(valT, io["value"], c.S)
    vaug = actpool.tile([P, c.ST, c.H, 66], BF16, tag="vaug")
    nc.vector.memset(vaug[:, :, :, 64:65], 1.0)
    hpd = c.DC // HD  # heads per d-chunk

    def vproj_ops(n, sc_list):
        """Thunks: 8 matmuls + eviction per s-tile for V d-chunk n."""
        ops = []
        for sc in sc_list:
            ps_holder = []
            for kc in range(c.KC):
                def mm(n=n, sc=sc, kc=kc, ps_holder=ps_holder):
                    if kc == 0:
                        ps_holder.append(proj_psum(c.DC))
                    nc.tensor.matmul(
                        ps_holder[-1][:],
                        lhsT=valT[:, kc, sc * P : (sc + 1) * P],
                        rhs=Wv_sb[:, kc, n * c.DC : (n + 1) * c.DC],
                        start=(kc == 0),
                        stop=(kc == c.KC - 1),
                    )
                    if kc == c.KC - 1:
                        nc.vector.tensor_copy(
                            out=vaug[:, sc, n * hpd : (n + 1) * hpd, 0:64],
                            in_=ps_holder[-1][:].rearrange(
                                "p (h x) -> p h x", x=HD
                            ),
                        )
                ops.append(mm)
        return ops

    # n=0 blocks are interleaved into pair 0's first t-chunk with a
    # 3-s-tile lookahead (PV of step st needs vaug[st]); nothing is
    # emitted before attention so the window opens right after K chunk 0
    for op in vproj_ops(0, list(range(c.ST))):
        op()

    # ---- attention ----
    OT = actpool.tile([P, c.PAIRS, c.T], BF16, tag="ot")
    for j in range(c.PAIRS):
        pending = []
        if j + 1 < c.KC:
            pending += kproj_chunk_ops(j + 1)
        if j < 4:
            pending += vproj_ops(1, list(range(4 * j, 4 * j + 4)))
        pend_i = 0
        Kt_j = kt_tiles[j]
        for n in range(c.NT):
            acc = psum.tile([65, c.AW], F32, name=f"acc_{j}_{n}", tag="acc")
            for st in range(c.ST):
                sp = psum.tile([P, c.AW], F32, name=f"sc_{j}_{n}_{st}", tag="sc",
                               bufs=2)
                for hh in range(2):
                    po = hh * HD
                    nc.tensor.matmul(
                        sp[:, hh * c.TC : (hh + 1) * c.TC],
                        lhsT=Kt_j[po : po + HD, st * P : (st + 1) * P],
                        rhs=Qt[po : po + HD, j, n * c.TC : (n + 1) * c.TC],
                        start=True,
                        stop=True,
                        tile_position=(po, 0),
                    )
                pt = ppool.tile([P, c.AW], BF16, tag="p")
                nc.scalar.activation(pt[:], sp[:], EXP, scale=float(scale))
                for hh in range(2):
                    sl = slice(hh * c.TC, (hh + 1) * c.TC)
                    nc.tensor.matmul(
                        acc[:, sl],
                        lhsT=vaug[:, st, 2 * j + hh, 0:65],
                        rhs=pt[:, sl],
                        start=(st == 0),
                        stop=(st == c.ST - 1),
                    )
                # interleave deferred projection work, two ops per step
                for _ in range(2):
                    if pend_i < len(pending):
                        pending[pend_i]()
                        pend_i += 1
            # normalization: O = O'[0:64]/l + bv ; l = O'[64]
            nrm = npool.tile([65, c.AW], F32, tag="nrm")
            nc.vector.tensor_copy(out=nrm[:], in_=acc[:])
            l_dram = dram.tile([c.AW], F32, name=f"ld_{j}_{n}", tag="ld", bufs=2)
            nc.sync.dma_start(out=l_dram[:], in_=nrm[64:65, :])
            rv = n1pool.tile([HD, c.AW], F32, name=f"rv_{j}_{n}", tag="rv")
            nc.sync.dma_start(out=rv[:], in_=_pbcast(l_dram[:], HD, drop_first=False))
            nc.vector.reciprocal_approx_fast(out=rv[:], in_=rv[:])
            tmp = n1pool.tile([HD, c.AW], BF16, name=f"tmp_{j}_{n}", tag="tmp")
            nc.vector.tensor_mul(out=tmp[:], in0=nrm[0:64, :], in1=rv[:])
            tsl = slice(n * c.TC, (n + 1) * c.TC)
            nc.vector.tensor_scalar_add(
                out=OT[0:64, j, tsl], in0=tmp[:, 0 : c.TC],
                scalar1=bv64[:, 2 * j : 2 * j + 1],
            )
            shf = n1pool.tile([HD, c.TC], BF16, name=f"shf_{j}_{n}", tag="shf")
            nc.vector.tensor_scalar_add(
                out=shf[:], in0=tmp[:, c.TC : 2 * c.TC],
                scalar1=bv64[:, 2 * j + 1 : 2 * j + 2],
            )
            nc.sync.dma_start(out=OT[64:128, j, tsl], in_=shf[:])
        while pend_i < len(pending):
            pending[pend_i]()
            pend_i += 1

    # ---- output projection (Wo load emitted here so its queue slot sits
    # after the attention bounces; its buffer frees once Wk is dead; casts
    # ride VectorE, which is idle in the tail, instead of ScalarE) ----
    Wo_sb = wpool.tile([P, c.KC, c.D], BF16, name="Wo_sb", tag="w")
    wo_src = io["Wo"][:].rearrange("(c p) n -> p c n", p=P)
    for kc in range(c.KC):
        for h in range(2):
            s = wstage.tile([P, c.D // 2], F32, tag="wst")
            eng = nc.sync if (2 * kc + h) % 2 == 0 else nc.scalar
            eng.dma_start(
                out=s[:],
                in_=wo_src[:, kc, h * (c.D // 2) : (h + 1) * (c.D // 2)],
            )
            nc.vector.tensor_copy(
                out=Wo_sb[:, kc, h * (c.D // 2) : (h + 1) * (c.D // 2)],
                in_=s[:],
            )
    for m in range(c.T // P):
        for n in range(c.ND):
            ps = proj_psum(c.DC)
            for j in range(c.PAIRS):
                nc.tensor.matmul(
                    ps[:],
                    lhsT=OT[:, j, m * P : (m + 1) * P],
                    rhs=Wo_sb[:, j, n * c.DC : (n + 1) * c.DC],
                    start=(j == 0),
                    stop=(j == c.PAIRS - 1),
                )
            osb = outpool.tile([P, c.DC], F32, tag="osb")
            nc.vector.tensor_add(
                out=osb[:], in0=ps[:], in1=bo_bc[:, n * c.DC : (n + 1) * c.DC]
            )
            nc.gpsimd.dma_start(
                out=io["out"][m * P : (m + 1) * P, n * c.DC : (n + 1) * c.DC],
                in_=osb[:],
            )


def build_nc(cfg=FULL):
    from contextlib import ExitStack

    nc = bacc.Bacc()
    io = {
        "query": nc.dram_tensor("query", [cfg.T, cfg.D], F32, kind="ExternalInput"),
        "key": nc.dram_tensor("key", [cfg.S, cfg.D], F32, kind="ExternalInput"),
        "value": nc.dram_tensor("value", [cfg.S, cfg.D], F32, kind="ExternalInput"),
        "Wq": nc.dram_tensor("Wq", [cfg.D, cfg.D], F32, kind="ExternalInput"),
        "Wk": nc.dram_tensor("Wk", [cfg.D, cfg.D], F32, kind="ExternalInput"),
        "Wv": nc.dram_tensor("Wv", [cfg.D, cfg.D], F32, kind="ExternalInput"),
        "Wo": nc.dram_tensor("Wo", [cfg.D, cfg.D], F32, kind="ExternalInput"),
        "bq": nc.dram_tensor("bq", [cfg.D], F32, kind="ExternalInput"),
        "bk": nc.dram_tensor("bk", [cfg.D], F32, kind="ExternalInput"),
        "bv": nc.dram_tensor("bv", [cfg.D], F32, kind="ExternalInput"),
        "bo": nc.dram_tensor("bo", [cfg.D], F32, kind="ExternalInput"),
        "out": nc.dram_tensor("out", [cfg.T, cfg.D], F32, kind="ExternalOutput"),
    }
    with tile.TileContext(nc) as tc:
        with ExitStack() as ctx:
            build_kernel(ctx, tc, cfg, io)
    nc.finalize()
    return nc



def make_in_maps(arr):
    """Per-core input maps for the 4-batch x 2-T-half sharding."""
    B, T_full, D = arr["query"].shape
    half = T_full // 2
    in_maps = []
    for core in range(N_CORES):
        b, h = divmod(core, 2)
        m = {
            "query": np.ascontiguousarray(arr["query"][b, h * half : (h + 1) * half]),
            "key": arr["key"][b],
            "value": arr["value"][b],
        }
        for w in ("Wq", "Wk", "Wv", "Wo", "bq", "bk", "bv", "bo"):
            m[w] = arr[w]
        in_maps.append(m)
    return in_maps


def run(inputs, trace=False):
    from concourse.bass_utils import run_bass_kernel_spmd

    arr = {k: np.ascontiguousarray(np.asarray(v, dtype=np.float32))
           for k, v in inputs.items()}
    B, T_full, D = arr["query"].shape
    half = T_full // 2
    nc = build_nc(FULL)
    in_maps = make_in_maps(arr)
    res = run_bass_kernel_spmd(nc, in_maps, list(range(N_CORES)), trace=trace)
    out = np.empty((B, T_full, D), np.float32)
    for core in range(N_CORES):
        b, h = divmod(core, 2)
        out[b, h * half : (h + 1) * half] = res.results[core]["out"]
    return out, res


def kernel(**inputs):
    out, _ = run(inputs, trace=False)
    return out
